# revision 1
# baseline (speedup 1.0000x reference)
"""Mixture-of-Experts (N=8192, D=2048, E=8, top-2) on 8 TRN2 NeuronCores.

Strategy (expert-parallel, per the sharding hint):
  - Routing (gate matmul + softmax + top-2) is computed on host with the
    exact same jax-on-CPU ops as the reference, so expert selection matches
    bitwise.  This is the "dispatch" part of the sharding strategy.
  - Core e receives: W_e^T (bf16, PE-friendly layout), the tokens routed to
    expert e (gathered + transposed + bf16, padded to a uniform capacity C),
    and b_e.  It computes yT = relu(W_e @ x_sel^T + b_e) with the tensor
    engine (bf16 matmuls, fp32 PSUM accumulation).
  - Host un-pads, applies the top-2 combine weights and scatter-adds the two
    expert contributions per token back into the full [N, D] fp32 output.

Layouts (P = 128 partitions):
  wt   [OT, P, KO, P] bf16 : wt[o, p, ko, q] = W_e[o*P+q, ko*P+p]
                             -> lhsT tile [:, o, ko] is [K=p, M=q]
  xt   [P, KO, C]     bf16 : xt[p, ko, c] = x_sel[c, ko*P+p]
  bias [P, OT]        f32  : bias[p, o] = b_e[o*P+p]
  yt   [P, OT, C]     f32  : yt[p, o, c] = y[c, o*P+p]
"""

import numpy as np
import ml_dtypes

N, D, E, TOP_K = 8192, 2048, 8, 2
P = 128
KO = D // P  # contraction tiles
OT = D // P  # output-channel tiles

# Set by a profiling harness (e.g. test.py) to get trace/exec_time_ns back.
PROFILE = False
LAST_RESULTS = None

_KERNEL_CACHE = {}


def _routing(x, W_gate, b_gate):
    """Top-2 gating, bitwise identical to the reference's jax-on-CPU math."""
    import jax
    import jax.numpy as jnp

    cpu = jax.devices("cpu")[0]
    with jax.default_device(cpu):
        xj = jax.device_put(np.asarray(x, dtype=np.float32), cpu)
        wg = jax.device_put(np.asarray(W_gate, dtype=np.float32), cpu)
        bg = jax.device_put(np.asarray(b_gate, dtype=np.float32), cpu)
        logits = xj @ wg.T + bg
        gate = jax.nn.softmax(logits, axis=-1)
        vals, idx = jax.lax.top_k(gate, TOP_K)
        vals, idx = np.asarray(vals), np.asarray(idx)
    return vals, idx


def _ctiles(C):
    tiles = []
    c0 = 0
    while c0 < C:
        cw = min(512, C - c0)
        tiles.append((c0, cw))
        c0 += cw
    return tuple(tiles)


def _build(C):
    """One-expert-per-core kernel: yt = relu(wt.T @ xt + bias)."""
    import concourse.tile as tile
    from concourse import bacc, mybir

    ctiles = _ctiles(C)
    nc = bacc.Bacc("TRN2", target_bir_lowering=False, debug=False)
    wt = nc.dram_tensor(
        "wt", [OT, P, KO, P], mybir.dt.bfloat16, kind="ExternalInput"
    ).ap()
    xt = nc.dram_tensor("xt", [P, KO, C], mybir.dt.bfloat16, kind="ExternalInput").ap()
    bias = nc.dram_tensor("bias", [P, OT], mybir.dt.float32, kind="ExternalInput").ap()
    yt = nc.dram_tensor("yt", [P, OT, C], mybir.dt.float32, kind="ExternalOutput").ap()

    with tile.TileContext(nc) as tc:
        with (
            tc.tile_pool(name="consts", bufs=1) as cpool,
            tc.tile_pool(name="outs", bufs=8) as opool,
            tc.tile_pool(name="psum", bufs=8, space="PSUM") as pspool,
        ):
            bias_sb = cpool.tile([P, OT], mybir.dt.float32)
            nc.sync.dma_start(bias_sb[:], bias[:])
            wt_sb = cpool.tile([P, OT, KO, P], mybir.dt.bfloat16)
            for o in range(OT):
                nc.sync.dma_start(wt_sb[:, o], wt[o])
            xt_sb = cpool.tile([P, KO, C], mybir.dt.bfloat16)
            for c0, cw in ctiles:
                nc.sync.dma_start(xt_sb[:, :, c0 : c0 + cw], xt[:, :, c0 : c0 + cw])

            for c0, cw in ctiles:
                for o in range(OT):
                    ps = pspool.tile([P, 512], mybir.dt.float32)
                    for ko in range(KO):
                        nc.tensor.matmul(
                            ps[:, :cw],
                            wt_sb[:, o, ko],
                            xt_sb[:, ko, c0 : c0 + cw],
                            start=(ko == 0),
                            stop=(ko == KO - 1),
                        )
                    ot = opool.tile([P, 512], mybir.dt.float32)
                    nc.scalar.activation(
                        ot[:, :cw],
                        ps[:, :cw],
                        mybir.ActivationFunctionType.Relu,
                        bias=bias_sb[:, o : o + 1],
                    )
                    nc.sync.dma_start(yt[:, o, c0 : c0 + cw], ot[:, :cw])
    nc.compile()
    return nc


def _get_kernel(C):
    if C not in _KERNEL_CACHE:
        _KERNEL_CACHE[C] = _build(C)
    return _KERNEL_CACHE[C]


def kernel(x, W_experts, b_experts, W_gate, b_gate):
    global LAST_RESULTS
    x = np.asarray(x, dtype=np.float32)
    W_experts = np.asarray(W_experts, dtype=np.float32)
    b_experts = np.asarray(b_experts, dtype=np.float32)

    vals, idx = _routing(x, W_gate, b_gate)

    sels, combine_w, counts = [], [], []
    for e in range(E):
        mask = idx == e  # [N, TOP_K]; an expert appears at most once per row
        rows = np.nonzero(mask.any(axis=1))[0]
        sels.append(rows)
        combine_w.append(vals[mask])  # row-major -> aligned with `rows`
        counts.append(len(rows))

    C = ((max(counts) + P - 1) // P) * P
    nc = _get_kernel(C)

    xbf = np.ascontiguousarray(x.astype(ml_dtypes.bfloat16))
    in_maps = []
    for e in range(E):
        cnt = counts[e]
        xe = np.zeros((P, KO, C), dtype=ml_dtypes.bfloat16)
        xsel = xbf[sels[e]]  # [cnt, D]
        xe[:, :, :cnt] = xsel.T.reshape(KO, P, cnt).transpose(1, 0, 2)
        we = np.ascontiguousarray(
            W_experts[e]
            .astype(ml_dtypes.bfloat16)
            .reshape(OT, P, KO, P)
            .transpose(0, 3, 2, 1)
        )
        be = np.ascontiguousarray(b_experts[e].reshape(OT, P).T)
        in_maps.append({"wt": we, "xt": xe, "bias": be})

    from concourse.bass_utils import run_bass_kernel_spmd

    res = run_bass_kernel_spmd(
        nc, in_maps, core_ids=list(range(E)), trace=PROFILE
    )
    LAST_RESULTS = res

    out = np.zeros((N, D), dtype=np.float32)
    for e in range(E):
        cnt = counts[e]
        yt_e = res.results[e]["yt"]  # [P, OT, C]
        y = yt_e[:, :, :cnt].transpose(2, 1, 0).reshape(cnt, D)
        out[sels[e]] += combine_w[e][:, None] * y
    return out


# revision 3
# speedup vs baseline: 1.1110x; 1.1110x over previous
"""Mixture-of-Experts (N=8192, D=2048, E=8, top-2) on 8 TRN2 NeuronCores.

Strategy (expert-parallel, per the sharding hint):
  - Routing (gate matmul + softmax + top-2) is computed on host with the
    exact same jax-on-CPU ops as the reference, so expert selection matches
    bitwise.  This is the "dispatch" part of the sharding strategy.
  - Core e receives: W_e^T (bf16, PE-friendly layout), the tokens routed to
    expert e (gathered + transposed + bf16, padded to a uniform capacity C),
    and b_e.  It computes yT = relu(W_e @ x_sel^T + b_e) with the tensor
    engine (bf16 matmuls, fp32 PSUM accumulation).
  - Host un-pads, applies the top-2 combine weights and scatter-adds the two
    expert contributions per token back into the full [N, D] fp32 output.

Layouts (P = 128 partitions):
  wt   [OT, P, KO, P] bf16 : wt[o, p, ko, q] = W_e[o*P+q, ko*P+p]
                             -> lhsT tile [:, o, ko] is [K=p, M=q]
  xt   [P, KO, C]     bf16 : xt[p, ko, c] = x_sel[c, ko*P+p]
  bias [P, OT]        f32  : bias[p, o] = b_e[o*P+p]
  yt   [P, OT, C]     f32  : yt[p, o, c] = y[c, o*P+p]
"""

import numpy as np
import ml_dtypes

N, D, E, TOP_K = 8192, 2048, 8, 2
P = 128
KO = D // P  # contraction tiles
OT = D // P  # output-channel tiles

# Set by a profiling harness (e.g. test.py) to get trace/exec_time_ns back.
PROFILE = False
LAST_RESULTS = None

_KERNEL_CACHE = {}


def _routing(x, W_gate, b_gate):
    """Top-2 gating, bitwise identical to the reference's jax-on-CPU math."""
    import jax
    import jax.numpy as jnp

    cpu = jax.devices("cpu")[0]
    with jax.default_device(cpu):
        xj = jax.device_put(np.asarray(x, dtype=np.float32), cpu)
        wg = jax.device_put(np.asarray(W_gate, dtype=np.float32), cpu)
        bg = jax.device_put(np.asarray(b_gate, dtype=np.float32), cpu)
        logits = xj @ wg.T + bg
        gate = jax.nn.softmax(logits, axis=-1)
        vals, idx = jax.lax.top_k(gate, TOP_K)
        vals, idx = np.asarray(vals), np.asarray(idx)
    return vals, idx


def _ctiles(C):
    tiles = []
    c0 = 0
    while c0 < C:
        cw = min(512, C - c0)
        tiles.append((c0, cw))
        c0 += cw
    return tuple(tiles)


def _build(C):
    """One-expert-per-core kernel: yt = relu(wt.T @ xt + bias)."""
    import concourse.tile as tile
    from concourse import bacc, mybir

    ctiles = _ctiles(C)
    nc = bacc.Bacc("TRN2", target_bir_lowering=False, debug=False)
    wt = nc.dram_tensor(
        "wt", [OT, P, KO, P], mybir.dt.bfloat16, kind="ExternalInput"
    ).ap()
    xt = nc.dram_tensor("xt", [P, KO, C], mybir.dt.bfloat16, kind="ExternalInput").ap()
    bias = nc.dram_tensor("bias", [P, OT], mybir.dt.float32, kind="ExternalInput").ap()
    yt = nc.dram_tensor("yt", [P, OT, C], mybir.dt.float32, kind="ExternalOutput").ap()

    with tile.TileContext(nc) as tc:
        with (
            tc.tile_pool(name="consts", bufs=1) as cpool,
            tc.tile_pool(name="outs", bufs=8) as opool,
            tc.tile_pool(name="psum", bufs=8, space="PSUM") as pspool,
        ):
            # Two HWDGE queues: xt streams on the sync-engine queue while
            # weights/bias (and later the outputs) go on the scalar-engine
            # queue.  Issue order is interleaved so the first (c0, o0) psum
            # group's operands land within a few microseconds — with a
            # single queue and all-weights-first ordering the PE sat idle
            # ~40us at the head waiting for its first inputs.
            bias_sb = cpool.tile([P, OT], mybir.dt.float32)
            nc.scalar.dma_start(bias_sb[:], bias[:])
            wt_sb = cpool.tile([P, OT, KO, P], mybir.dt.bfloat16)
            xt_sb = cpool.tile([P, KO, C], mybir.dt.bfloat16)

            def load_wt(o, ksplit=1):
                kc = KO // ksplit
                for i in range(ksplit):
                    nc.scalar.dma_start(
                        wt_sb[:, o, i * kc : (i + 1) * kc],
                        wt[o, :, i * kc : (i + 1) * kc],
                    )

            def load_xt(ci, ksplit=1):
                c0, cw = ctiles[ci]
                kc = KO // ksplit
                for i in range(ksplit):
                    nc.sync.dma_start(
                        xt_sb[:, i * kc : (i + 1) * kc, c0 : c0 + cw],
                        xt[:, i * kc : (i + 1) * kc, c0 : c0 + cw],
                    )

            load_wt(0, ksplit=4)
            load_xt(0, ksplit=4)
            load_wt(1, ksplit=2)
            if len(ctiles) > 1:
                load_xt(1, ksplit=2)
            for o in range(2, OT):
                load_wt(o)
            for ci in range(2, len(ctiles)):
                load_xt(ci)

            for c0, cw in ctiles:
                for o in range(OT):
                    ps = pspool.tile([P, 512], mybir.dt.float32)
                    for ko in range(KO):
                        nc.tensor.matmul(
                            ps[:, :cw],
                            wt_sb[:, o, ko],
                            xt_sb[:, ko, c0 : c0 + cw],
                            start=(ko == 0),
                            stop=(ko == KO - 1),
                        )
                    ot = opool.tile([P, 512], mybir.dt.float32)
                    nc.scalar.activation(
                        ot[:, :cw],
                        ps[:, :cw],
                        mybir.ActivationFunctionType.Relu,
                        bias=bias_sb[:, o : o + 1],
                    )
                    nc.scalar.dma_start(yt[:, o, c0 : c0 + cw], ot[:, :cw])
    nc.compile()
    return nc


def _get_kernel(C):
    if C not in _KERNEL_CACHE:
        _KERNEL_CACHE[C] = _build(C)
    return _KERNEL_CACHE[C]


def kernel(x, W_experts, b_experts, W_gate, b_gate):
    global LAST_RESULTS
    x = np.asarray(x, dtype=np.float32)
    W_experts = np.asarray(W_experts, dtype=np.float32)
    b_experts = np.asarray(b_experts, dtype=np.float32)

    vals, idx = _routing(x, W_gate, b_gate)

    sels, combine_w, counts = [], [], []
    for e in range(E):
        mask = idx == e  # [N, TOP_K]; an expert appears at most once per row
        rows = np.nonzero(mask.any(axis=1))[0]
        sels.append(rows)
        combine_w.append(vals[mask])  # row-major -> aligned with `rows`
        counts.append(len(rows))

    C = ((max(counts) + 3) // 4) * 4  # exact capacity, 8B-aligned rows
    nc = _get_kernel(C)

    xbf = np.ascontiguousarray(x.astype(ml_dtypes.bfloat16))
    in_maps = []
    for e in range(E):
        cnt = counts[e]
        xe = np.zeros((P, KO, C), dtype=ml_dtypes.bfloat16)
        xsel = xbf[sels[e]]  # [cnt, D]
        xe[:, :, :cnt] = xsel.T.reshape(KO, P, cnt).transpose(1, 0, 2)
        we = np.ascontiguousarray(
            W_experts[e]
            .astype(ml_dtypes.bfloat16)
            .reshape(OT, P, KO, P)
            .transpose(0, 3, 2, 1)
        )
        be = np.ascontiguousarray(b_experts[e].reshape(OT, P).T)
        in_maps.append({"wt": we, "xt": xe, "bias": be})

    from concourse.bass_utils import run_bass_kernel_spmd

    res = run_bass_kernel_spmd(
        nc, in_maps, core_ids=list(range(E)), trace=PROFILE
    )
    LAST_RESULTS = res

    out = np.zeros((N, D), dtype=np.float32)
    for e in range(E):
        cnt = counts[e]
        yt_e = res.results[e]["yt"]  # [P, OT, C]
        y = yt_e[:, :, :cnt].transpose(2, 1, 0).reshape(cnt, D)
        out[sels[e]] += combine_w[e][:, None] * y
    return out


# revision 5
# speedup vs baseline: 1.1284x; 1.0157x over previous
"""Mixture-of-Experts (N=8192, D=2048, E=8, top-2) on 8 TRN2 NeuronCores.

Strategy (expert-parallel, per the sharding hint):
  - Routing (gate matmul + softmax + top-2) is computed on host with the
    exact same jax-on-CPU ops as the reference, so expert selection matches
    bitwise.  This is the "dispatch" part of the sharding strategy.
  - Core e receives: W_e^T (bf16, PE-friendly layout), the tokens routed to
    expert e (gathered + transposed + bf16, padded to a uniform capacity C),
    and b_e.  It computes yT = relu(W_e @ x_sel^T + b_e) with the tensor
    engine (bf16 matmuls, fp32 PSUM accumulation).
  - Host un-pads, applies the top-2 combine weights and scatter-adds the two
    expert contributions per token back into the full [N, D] fp32 output.

Layouts (P = 128 partitions):
  wt   [OT, P, KO, P] bf16 : wt[o, p, ko, q] = W_e[o*P+q, ko*P+p]
                             -> lhsT tile [:, o, ko] is [K=p, M=q]
  xt   [P, KO, C]     bf16 : xt[p, ko, c] = x_sel[c, ko*P+p]
  bias [P, OT]        f32  : bias[p, o] = b_e[o*P+p]
  yt   [P, OT, C]     f32  : yt[p, o, c] = y[c, o*P+p]
"""

import numpy as np
import ml_dtypes

N, D, E, TOP_K = 8192, 2048, 8, 2
P = 128
KO = D // P  # contraction tiles
OT = D // P  # output-channel tiles

# Set by a profiling harness (e.g. test.py) to get trace/exec_time_ns back.
PROFILE = False
LAST_RESULTS = None

_KERNEL_CACHE = {}


def _routing(x, W_gate, b_gate):
    """Top-2 gating, bitwise identical to the reference's jax-on-CPU math."""
    import jax
    import jax.numpy as jnp

    cpu = jax.devices("cpu")[0]
    with jax.default_device(cpu):
        xj = jax.device_put(np.asarray(x, dtype=np.float32), cpu)
        wg = jax.device_put(np.asarray(W_gate, dtype=np.float32), cpu)
        bg = jax.device_put(np.asarray(b_gate, dtype=np.float32), cpu)
        logits = xj @ wg.T + bg
        gate = jax.nn.softmax(logits, axis=-1)
        vals, idx = jax.lax.top_k(gate, TOP_K)
        vals, idx = np.asarray(vals), np.asarray(idx)
    return vals, idx


def _ctiles(C):
    # Tile widths must stay >=256 where possible: below that the per-matmul
    # LDWEIGHTS (~107ns) exceeds the N/2.4GHz stream time and the PE becomes
    # weight-load-bound.  Take 512s while >1024 remains, then split the
    # remainder into two near-equal tiles (each >=256 whenever C%512 != 0).
    widths = []
    rem = C
    while rem > 1024:
        widths.append(512)
        rem -= 512
    if rem > 512:
        widths.extend([(rem + 1) // 2, rem // 2])
    elif rem:
        widths.append(rem)
    tiles, c0 = [], 0
    for w in widths:
        tiles.append((c0, w))
        c0 += w
    return tuple(tiles)


def _build(C):
    """One-expert-per-core kernel: yt = relu(wt.T @ xt + bias)."""
    import concourse.tile as tile
    from concourse import bacc, mybir

    ctiles = _ctiles(C)
    nc = bacc.Bacc("TRN2", target_bir_lowering=False, debug=False)
    wt = nc.dram_tensor(
        "wt", [OT, P, KO, P], mybir.dt.bfloat16, kind="ExternalInput"
    ).ap()
    xt = nc.dram_tensor("xt", [P, KO, C], mybir.dt.bfloat16, kind="ExternalInput").ap()
    bias = nc.dram_tensor("bias", [P, OT], mybir.dt.float32, kind="ExternalInput").ap()
    yt = nc.dram_tensor("yt", [P, OT, C], mybir.dt.float32, kind="ExternalOutput").ap()

    with tile.TileContext(nc) as tc:
        with (
            tc.tile_pool(name="consts", bufs=1) as cpool,
            tc.tile_pool(name="outs", bufs=8) as opool,
            tc.tile_pool(name="psum", bufs=8, space="PSUM") as pspool,
        ):
            # Engine/queue assignment (both HBM streams share ~360GB/s, so
            # ordering matters more than queue count):
            #   scalar HWDGE: bias + ALL weights (needed at a steady
            #     0.53MB/3.4us from the start), then the later xt chunks.
            #   sync HWDGE: first xt chunk (head-critical), then the 80
            #     per-group output stores (sync engine is otherwise idle).
            #   vector engine: psum eviction = fused bias-add + relu
            #     (tensor_scalar), keeping ACT/sync sequencers free.
            bias_sb = cpool.tile([P, OT], mybir.dt.float32)
            nc.scalar.dma_start(bias_sb[:], bias[:])
            wt_sb = cpool.tile([P, OT, KO, P], mybir.dt.bfloat16)
            xt_sb = cpool.tile([P, KO, C], mybir.dt.bfloat16)

            def load_wt(o, ksplit=1):
                kc = KO // ksplit
                for i in range(ksplit):
                    nc.scalar.dma_start(
                        wt_sb[:, o, i * kc : (i + 1) * kc],
                        wt[o, :, i * kc : (i + 1) * kc],
                    )

            def load_xt(ci, engine, ksplit=1):
                c0, cw = ctiles[ci]
                kc = KO // ksplit
                for i in range(ksplit):
                    engine.dma_start(
                        xt_sb[:, i * kc : (i + 1) * kc, c0 : c0 + cw],
                        xt[:, i * kc : (i + 1) * kc, c0 : c0 + cw],
                    )

            load_xt(0, nc.sync, ksplit=4)
            load_wt(0, ksplit=4)
            load_wt(1, ksplit=2)
            for o in range(2, OT):
                load_wt(o)
            for ci in range(1, len(ctiles)):
                load_xt(ci, nc.scalar)

            for c0, cw in ctiles:
                for o in range(OT):
                    ps = pspool.tile([P, 512], mybir.dt.float32)
                    for ko in range(KO):
                        nc.tensor.matmul(
                            ps[:, :cw],
                            wt_sb[:, o, ko],
                            xt_sb[:, ko, c0 : c0 + cw],
                            start=(ko == 0),
                            stop=(ko == KO - 1),
                        )
                    ot = opool.tile([P, 512], mybir.dt.float32)
                    # ot = max(ps + bias, 0) on the vector engine
                    nc.vector.tensor_scalar(
                        ot[:, :cw],
                        ps[:, :cw],
                        bias_sb[:, o : o + 1],
                        0.0,
                        mybir.AluOpType.add,
                        mybir.AluOpType.max,
                    )
                    nc.sync.dma_start(yt[:, o, c0 : c0 + cw], ot[:, :cw])
    nc.compile()
    return nc


def _get_kernel(C):
    if C not in _KERNEL_CACHE:
        _KERNEL_CACHE[C] = _build(C)
    return _KERNEL_CACHE[C]


def kernel(x, W_experts, b_experts, W_gate, b_gate):
    global LAST_RESULTS
    x = np.asarray(x, dtype=np.float32)
    W_experts = np.asarray(W_experts, dtype=np.float32)
    b_experts = np.asarray(b_experts, dtype=np.float32)

    vals, idx = _routing(x, W_gate, b_gate)

    sels, combine_w, counts = [], [], []
    for e in range(E):
        mask = idx == e  # [N, TOP_K]; an expert appears at most once per row
        rows = np.nonzero(mask.any(axis=1))[0]
        sels.append(rows)
        combine_w.append(vals[mask])  # row-major -> aligned with `rows`
        counts.append(len(rows))

    C = ((max(counts) + 3) // 4) * 4  # exact capacity, 8B-aligned rows
    nc = _get_kernel(C)

    xbf = np.ascontiguousarray(x.astype(ml_dtypes.bfloat16))
    in_maps = []
    for e in range(E):
        cnt = counts[e]
        xe = np.zeros((P, KO, C), dtype=ml_dtypes.bfloat16)
        xsel = xbf[sels[e]]  # [cnt, D]
        xe[:, :, :cnt] = xsel.T.reshape(KO, P, cnt).transpose(1, 0, 2)
        we = np.ascontiguousarray(
            W_experts[e]
            .astype(ml_dtypes.bfloat16)
            .reshape(OT, P, KO, P)
            .transpose(0, 3, 2, 1)
        )
        be = np.ascontiguousarray(b_experts[e].reshape(OT, P).T)
        in_maps.append({"wt": we, "xt": xe, "bias": be})

    from concourse.bass_utils import run_bass_kernel_spmd

    res = run_bass_kernel_spmd(
        nc, in_maps, core_ids=list(range(E)), trace=PROFILE
    )
    LAST_RESULTS = res

    out = np.zeros((N, D), dtype=np.float32)
    for e in range(E):
        cnt = counts[e]
        yt_e = res.results[e]["yt"]  # [P, OT, C]
        y = yt_e[:, :, :cnt].transpose(2, 1, 0).reshape(cnt, D)
        out[sels[e]] += combine_w[e][:, None] * y
    return out


# revision 11
# speedup vs baseline: 1.1700x; 1.0369x over previous
"""Mixture-of-Experts (N=8192, D=2048, E=8, top-2) on 8 TRN2 NeuronCores.

Strategy (expert-parallel, per the sharding hint):
  - Routing (gate matmul + softmax + top-2) is computed on host with the
    exact same jax-on-CPU ops as the reference, so expert selection matches
    bitwise.  This is the "dispatch" part of the sharding strategy.
  - Core e receives: W_e^T (bf16, PE-friendly layout), the tokens routed to
    expert e (gathered + transposed + bf16, padded to a uniform capacity C),
    and b_e.  It computes yT = relu(W_e @ x_sel^T + b_e) with the tensor
    engine (bf16 matmuls, fp32 PSUM accumulation).
  - Host un-pads, applies the top-2 combine weights and scatter-adds the two
    expert contributions per token back into the full [N, D] fp32 output.

Layouts (P = 128 partitions):
  wt   [OT, P, KO, P] bf16 : wt[o, p, ko, q] = W_e[o*P+q, ko*P+p]
                             -> lhsT tile [:, o, ko] is [K=p, M=q]
  xt   [P, KO, C]     bf16 : xt[p, ko, c] = x_sel[c, ko*P+p]
  bias [P, OT]        f32  : bias[p, o] = b_e[o*P+p]
  yt   [P, OT, C]     f32  : yt[p, o, c] = y[c, o*P+p]
"""

import numpy as np
import ml_dtypes

N, D, E, TOP_K = 8192, 2048, 8, 2
P = 128
KO = D // P  # contraction tiles
OT = D // P  # output-channel tiles

# Set by a profiling harness (e.g. test.py) to get trace/exec_time_ns back.
PROFILE = False
LAST_RESULTS = None

_KERNEL_CACHE = {}


def _routing(x, W_gate, b_gate):
    """Top-2 gating, bitwise identical to the reference's jax-on-CPU math."""
    import jax
    import jax.numpy as jnp

    cpu = jax.devices("cpu")[0]
    with jax.default_device(cpu):
        xj = jax.device_put(np.asarray(x, dtype=np.float32), cpu)
        wg = jax.device_put(np.asarray(W_gate, dtype=np.float32), cpu)
        bg = jax.device_put(np.asarray(b_gate, dtype=np.float32), cpu)
        logits = xj @ wg.T + bg
        gate = jax.nn.softmax(logits, axis=-1)
        vals, idx = jax.lax.top_k(gate, TOP_K)
        vals, idx = np.asarray(vals), np.asarray(idx)
    return vals, idx


def _ctiles(C):
    # Tile widths must stay >=256 where possible: below that the per-matmul
    # LDWEIGHTS (~107ns) exceeds the N/2.4GHz stream time and the PE becomes
    # weight-load-bound.  Take 512s while >1024 remains, then split the
    # remainder into two near-equal tiles (each >=256 whenever C%512 != 0).
    widths = []
    rem = C
    while rem > 1024:
        widths.append(512)
        rem -= 512
    if rem > 512:
        widths.extend([(rem + 1) // 2, rem // 2])
    elif rem:
        widths.append(rem)
    tiles, c0 = [], 0
    for w in widths:
        tiles.append((c0, w))
        c0 += w
    return tuple(tiles)


def _build(C):
    """One-expert-per-core kernel: yt = relu(wt.T @ xt + bias)."""
    import concourse.tile as tile
    from concourse import bacc, mybir

    ctiles = _ctiles(C)
    nc = bacc.Bacc("TRN2", target_bir_lowering=False, debug=False)
    wt = nc.dram_tensor(
        "wt", [OT, P, KO, P], mybir.dt.bfloat16, kind="ExternalInput"
    ).ap()
    xt = nc.dram_tensor("xt", [P, KO, C], mybir.dt.bfloat16, kind="ExternalInput").ap()
    bias = nc.dram_tensor("bias", [P, OT], mybir.dt.float32, kind="ExternalInput").ap()
    yt = nc.dram_tensor("yt", [P, OT, C], mybir.dt.bfloat16, kind="ExternalOutput").ap()

    with tile.TileContext(nc) as tc:
        with (
            tc.tile_pool(name="consts", bufs=1) as cpool,
            tc.tile_pool(name="outs", bufs=12) as opool,
            tc.tile_pool(name="psum", bufs=8, space="PSUM") as pspool,
        ):
            # Engine/queue assignment (both HBM streams share ~360GB/s, so
            # ordering matters more than queue count):
            #   scalar HWDGE: bias + ALL weights (needed at a steady
            #     0.53MB/3.4us from the start), then the later xt chunks.
            #   sync HWDGE: first xt chunk (head-critical), then the 80
            #     per-group output stores (sync engine is otherwise idle).
            #   vector engine: psum eviction = fused bias-add + relu
            #     (tensor_scalar), keeping ACT/sync sequencers free.
            bias_sb = cpool.tile([P, OT], mybir.dt.float32)
            wt_sb = cpool.tile([P, OT, KO, P], mybir.dt.bfloat16)
            xt_sb = cpool.tile([P, KO, C], mybir.dt.bfloat16)

            def load_wt(o, ksplit=1):
                kc = KO // ksplit
                for i in range(ksplit):
                    nc.scalar.dma_start(
                        wt_sb[:, o, i * kc : (i + 1) * kc],
                        wt[o, :, i * kc : (i + 1) * kc],
                    )

            def load_xt(ci, engine, ksplit=1):
                c0, cw = ctiles[ci]
                kc = KO // ksplit
                for i in range(ksplit):
                    engine.dma_start(
                        xt_sb[:, i * kc : (i + 1) * kc, c0 : c0 + cw],
                        xt[:, i * kc : (i + 1) * kc, c0 : c0 + cw],
                    )

            load_xt(0, nc.sync, ksplit=2)
            load_wt(0, ksplit=2)
            load_wt(1)
            nc.scalar.dma_start(bias_sb[:], bias[:])
            for o in range(2, OT):
                load_wt(o)
            for ci in range(1, len(ctiles)):
                load_xt(ci, nc.scalar)

            group = 0
            for c0, cw in ctiles:
                for o in range(OT):
                    ps = pspool.tile([P, 512], mybir.dt.float32)
                    for ko in range(KO):
                        nc.tensor.matmul(
                            ps[:, :cw],
                            wt_sb[:, o, ko],
                            xt_sb[:, ko, c0 : c0 + cw],
                            start=(ko == 0),
                            stop=(ko == KO - 1),
                        )
                    ot = opool.tile([P, 512], mybir.dt.bfloat16)
                    # ot = max(ps + bias, 0) on the vector engine
                    nc.vector.tensor_scalar(
                        ot[:, :cw],
                        ps[:, :cw],
                        bias_sb[:, o : o + 1],
                        0.0,
                        mybir.AluOpType.add,
                        mybir.AluOpType.max,
                    )
                    out_eng = nc.sync if group % 2 == 0 else nc.scalar
                    out_eng.dma_start(yt[:, o, c0 : c0 + cw], ot[:, :cw])
                    group += 1
    nc.compile()
    return nc


def _get_kernel(C):
    if C not in _KERNEL_CACHE:
        _KERNEL_CACHE[C] = _build(C)
    return _KERNEL_CACHE[C]


def kernel(x, W_experts, b_experts, W_gate, b_gate):
    global LAST_RESULTS
    x = np.asarray(x, dtype=np.float32)
    W_experts = np.asarray(W_experts, dtype=np.float32)
    b_experts = np.asarray(b_experts, dtype=np.float32)

    vals, idx = _routing(x, W_gate, b_gate)

    sels, combine_w, counts = [], [], []
    for e in range(E):
        mask = idx == e  # [N, TOP_K]; an expert appears at most once per row
        rows = np.nonzero(mask.any(axis=1))[0]
        sels.append(rows)
        combine_w.append(vals[mask])  # row-major -> aligned with `rows`
        counts.append(len(rows))

    C = ((max(counts) + 3) // 4) * 4  # exact capacity, 8B-aligned rows
    nc = _get_kernel(C)

    xbf = np.ascontiguousarray(x.astype(ml_dtypes.bfloat16))
    in_maps = []
    for e in range(E):
        cnt = counts[e]
        xe = np.zeros((P, KO, C), dtype=ml_dtypes.bfloat16)
        xsel = xbf[sels[e]]  # [cnt, D]
        xe[:, :, :cnt] = xsel.T.reshape(KO, P, cnt).transpose(1, 0, 2)
        we = np.ascontiguousarray(
            W_experts[e]
            .astype(ml_dtypes.bfloat16)
            .reshape(OT, P, KO, P)
            .transpose(0, 3, 2, 1)
        )
        be = np.ascontiguousarray(b_experts[e].reshape(OT, P).T)
        in_maps.append({"wt": we, "xt": xe, "bias": be})

    from concourse.bass_utils import run_bass_kernel_spmd

    res = run_bass_kernel_spmd(
        nc, in_maps, core_ids=list(range(E)), trace=PROFILE
    )
    LAST_RESULTS = res

    out = np.zeros((N, D), dtype=np.float32)
    for e in range(E):
        cnt = counts[e]
        yt_e = res.results[e]["yt"]  # [P, OT, C] bf16
        y = yt_e[:, :, :cnt].astype(np.float32).transpose(2, 1, 0).reshape(cnt, D)
        out[sels[e]] += combine_w[e][:, None] * y
    return out
